# revision 6
# baseline (speedup 1.0000x reference)
"""NeuralSDE forecasting kernel for 8x Trainium2 NeuronCores (Bass/Tile).

Data-parallel over batch B=256 across 8 cores (32 batch elems per core).
Per-core scan runs feature-major ("transposed") so no transposes are needed:
state y.T is stored as a [128 partitions, 4*32] SBUF tile where column block
k holds features 128k..128k+128 for the 32 local batch elements.

Matmuls are orientation-2: out = lhsT.T @ rhs with the (constant) weight
tiles as the stationary operand and the state as the 32-column moving
operand, accumulating over 4 K-chunks (+1 "x-tilde" chunk carrying the
control input and the bias via an appended ones-row). Everything is fp32:
the 255-step recurrence amplifies rounding noise ~5000x, so bf16/tf32
operands fail accuracy (measured 0.19 rel err for bf16).

sigmoid(x) is computed as 0.5*(1+tanh(x/2)) so the scan only ever needs the
Tanh ACT table; the 0.5 factors are folded into the host-prescaled dW.
"""

import sys

sys.path.insert(0, "/opt/trn_rl_repo")

import numpy as np

import concourse.bass as bass
import concourse.bacc as bacc
import concourse.mybir as mybir
import concourse.tile as tile
from concourse.bass_utils import run_bass_kernel_spmd

import os

B, T, C, H, O = 256, 256, 32, 512, 32
OUT_TIME = 32
NCORES = 8
BL = B // NCORES  # 32 batch elements per core
NT = int(os.environ.get("BASS_NT", T - 1))  # 255 scan steps
SAVE0 = NT - OUT_TIME  # first step whose y_next lands in the output tail
KC = H // 128  # 4 feature chunks
F32 = mybir.dt.float32

Tanh = mybir.ActivationFunctionType.Tanh
Relu = mybir.ActivationFunctionType.Relu
Identity = mybir.ActivationFunctionType.Identity

_BUILT = None  # cached (nc,) build


def _build_nc():
    nc = bacc.Bacc("TRN2", target_bir_lowering=False, debug=False)

    # --- DRAM I/O (per-core shards; weights replicated) ---
    d_xt = nc.dram_tensor("xt", [T, C + 1, BL], F32, kind="ExternalInput")
    d_dw = nc.dram_tensor("dw", [NT, 128, KC * BL], F32, kind="ExternalInput")
    d_w1y = nc.dram_tensor("w1y", [128, KC * H], F32, kind="ExternalInput")
    d_w2 = nc.dram_tensor("w2", [128, KC * H], F32, kind="ExternalInput")
    d_wg = nc.dram_tensor("wg", [128, KC * H], F32, kind="ExternalInput")
    d_wh1 = nc.dram_tensor("wh1", [128, KC * H], F32, kind="ExternalInput")
    d_w1b = nc.dram_tensor("w1b", [C + 1, H], F32, kind="ExternalInput")
    d_b2x = nc.dram_tensor("b2x", [C + 1, H], F32, kind="ExternalInput")
    d_bgx = nc.dram_tensor("bgx", [C + 1, H], F32, kind="ExternalInput")
    d_wini = nc.dram_tensor("wini", [C + 1, H], F32, kind="ExternalInput")
    d_wh2 = nc.dram_tensor("wh2", [128, KC * O], F32, kind="ExternalInput")
    d_bh1 = nc.dram_tensor("bh1t", [128, KC], F32, kind="ExternalInput")
    d_bh2 = nc.dram_tensor("bh2t", [O, 1], F32, kind="ExternalInput")
    d_out = nc.dram_tensor("out", [O, OUT_TIME * BL], F32, kind="ExternalOutput")

    with tile.TileContext(nc) as tc:
        with (
            tc.tile_pool(name="const", bufs=1) as const,
            tc.tile_pool(name="xp", bufs=6) as xp,
            tc.tile_pool(name="dwp", bufs=6) as dwp,
            tc.tile_pool(name="yp", bufs=2) as yp,
            tc.tile_pool(name="tmp", bufs=3) as tmp,
            tc.tile_pool(name="pp", bufs=2, space="PSUM") as pp,
        ):
            # --- resident weights ---
            w1y = const.tile([128, KC * H], F32, tag="w1y")
            w2 = const.tile([128, KC * H], F32, tag="w2")
            wg = const.tile([128, KC * H], F32, tag="wg")
            wh1 = const.tile([128, KC * H], F32, tag="wh1")
            w1b = const.tile([C + 1, H], F32, tag="w1b")
            b2x = const.tile([C + 1, H], F32, tag="b2x")
            bgx = const.tile([C + 1, H], F32, tag="bgx")
            wini = const.tile([C + 1, H], F32, tag="wini")
            wh2 = const.tile([128, KC * O], F32, tag="wh2")
            bh1 = const.tile([128, KC], F32, tag="bh1")
            bh2 = const.tile([O, 1], F32, tag="bh2")
            slab = const.tile([128, OUT_TIME * 128], F32, tag="slab")
            rT = const.tile([128, KC * 1024], F32, tag="rT")
            outs = const.tile([O, OUT_TIME * BL], F32, tag="outs")
            for dst, src in [
                (w1y, d_w1y), (w2, d_w2), (wg, d_wg), (wh1, d_wh1),
                (w1b, d_w1b), (b2x, d_b2x), (bgx, d_bgx), (wini, d_wini),
                (wh2, d_wh2), (bh1, d_bh1), (bh2, d_bh2),
            ]:
                nc.sync.dma_start(out=dst[:], in_=src[:])

            def wslice(w, k, m):
                return w[:, k * H + m * 128 : k * H + (m + 1) * 128]

            # --- z0 = x~_0 @ [W_init; b_init]/dt ---
            x0 = xp.tile([C + 1, BL], F32, tag="x")
            nc.sync.dma_start(out=x0[:], in_=d_xt[0])
            ps0 = pp.tile([128, 128], F32, tag="psA")
            for m in range(KC):
                nc.tensor.matmul(
                    ps0[:, m * BL : (m + 1) * BL],
                    wini[:, m * 128 : (m + 1) * 128],
                    x0[:],
                    start=True,
                    stop=True,
                )
            y_t = yp.tile([128, KC * BL], F32, tag="y")
            nc.vector.tensor_copy(y_t[:], ps0[:])
            y = y_t[:]

            # --- scan ---
            for t in range(NT):
                x_t = xp.tile([C + 1, BL], F32, tag="x")
                nc.sync.dma_start(out=x_t[:], in_=d_xt[t])
                dw_t = dwp.tile([128, KC * BL], F32, tag="dw")
                nc.sync.dma_start(out=dw_t[:], in_=d_dw[t])

                # h = tanh(y@W1y + x@W1x + b1)   (feature-major, psA)
                psA = pp.tile([128, 128], F32, tag="psA")
                for m in range(KC):
                    om = psA[:, m * BL : (m + 1) * BL]
                    nc.tensor.matmul(
                        om, w1b[:, m * 128 : (m + 1) * 128], x_t[:],
                        start=True, stop=False,
                    )
                    for k in range(KC):
                        nc.tensor.matmul(
                            om, wslice(w1y, k, m), y[:, k * BL : (k + 1) * BL],
                            start=False, stop=(k == KC - 1),
                        )
                h = tmp.tile([128, KC * BL], F32, tag="h")
                nc.scalar.activation(h[:], psA[:], Tanh)

                # tau = tanh((y@Wg + bg)/2); sigmoid fold (psC)
                psC = pp.tile([128, 128], F32, tag="psC")
                for m in range(KC):
                    om = psC[:, m * BL : (m + 1) * BL]
                    nc.tensor.matmul(
                        om, bgx[:, m * 128 : (m + 1) * 128], x_t[:],
                        start=True, stop=False,
                    )
                    for k in range(KC):
                        nc.tensor.matmul(
                            om, wslice(wg, k, m), y[:, k * BL : (k + 1) * BL],
                            start=False, stop=(k == KC - 1),
                        )
                tau = tmp.tile([128, KC * BL], F32, tag="tau")
                nc.scalar.activation(tau[:], psC[:], Tanh, scale=0.5)
                # t1 = (tau + 1) * dw_scaled ;  dw pre-scaled by 0.5*sqrt(dt)/dt
                t1 = tmp.tile([128, KC * BL], F32, tag="t1")
                nc.vector.scalar_tensor_tensor(
                    t1[:], tau[:], 1.0, dw_t[:],
                    mybir.AluOpType.add, mybir.AluOpType.mult,
                )
                yh = tmp.tile([128, KC * BL], F32, tag="yh")
                nc.vector.tensor_add(yh[:], y, t1[:])

                # f = tanh(h@W2 + b2)   (psB)
                psB = pp.tile([128, 128], F32, tag="psB")
                for m in range(KC):
                    om = psB[:, m * BL : (m + 1) * BL]
                    nc.tensor.matmul(
                        om, b2x[:, m * 128 : (m + 1) * 128], x_t[:],
                        start=True, stop=False,
                    )
                    for k in range(KC):
                        nc.tensor.matmul(
                            om, wslice(w2, k, m), h[:, k * BL : (k + 1) * BL],
                            start=False, stop=(k == KC - 1),
                        )
                f = tmp.tile([128, KC * BL], F32, tag="f")
                nc.scalar.activation(f[:], psB[:], Tanh)

                # y_next = y + t1 + f ; tail states land directly in the slab
                if t >= SAVE0:
                    y2 = slab[:, (t - SAVE0) * 128 : (t - SAVE0 + 1) * 128]
                else:
                    y2_t = yp.tile([128, KC * BL], F32, tag="y", name=f"y_{t}")
                    y2 = y2_t[:]
                nc.vector.tensor_add(y2, yh[:], f[:])
                y = y2

            # --- head: out = relu(z_tail@Wh1 + bh1) @ Wh2 + bh2 ---
            # slab columns: s*128 + k*32 + b  (s = tail step, k = feat chunk)
            slab_r = slab[:].rearrange(
                "p (s k b) -> p s k b", s=OUT_TIME, k=KC, b=BL
            )
            for m in range(KC):
                for hf in range(2):
                    ps1 = pp.tile([128, 512], F32, tag="psA")
                    for k in range(KC):
                        nc.tensor.matmul(
                            ps1[:],
                            wslice(wh1, k, m),
                            slab_r[:, hf * 16 : (hf + 1) * 16, k, :],
                            start=(k == 0),
                            stop=(k == KC - 1),
                        )
                    nc.scalar.activation(
                        rT[:, m * 1024 + hf * 512 : m * 1024 + (hf + 1) * 512],
                        ps1[:],
                        Relu,
                        bias=bh1[:, m : m + 1],
                    )
            for hf in range(2):
                ps2 = pp.tile([O, 512], F32, tag="psB")
                for m in range(KC):
                    nc.tensor.matmul(
                        ps2[:],
                        wh2[:, m * O : (m + 1) * O],
                        rT[:, m * 1024 + hf * 512 : m * 1024 + (hf + 1) * 512],
                        start=(m == 0),
                        stop=(m == KC - 1),
                    )
                nc.scalar.activation(
                    outs[:, hf * 512 : (hf + 1) * 512],
                    ps2[:],
                    Identity,
                    bias=bh2[:],
                )
            nc.sync.dma_start(out=d_out[:], in_=outs[:])

    nc.compile()
    return nc


def _prep_inputs(times, coeffs, final_index, dW, W_init, b_init, W1, b1, W2,
                 b2, Wg, bg, Wh1, bh1, Wh2, bh2):
    """Host-side reshape/prescale into the kernel's layouts. Returns
    (shared weight map, per-core input maps)."""
    f32 = np.float32
    times = np.asarray(times, f32)
    dt = f32(max(np.min(times[1:] - times[:-1]), f32(0.001)))
    sq = f32(np.sqrt(dt))

    def lhsT_layout(w):  # [H, H] -> [128, KC*H] with (k,m) tile at k*H+m*128
        return np.ascontiguousarray(
            np.asarray(w, f32).reshape(KC, 128, H).transpose(1, 0, 2).reshape(128, KC * H)
        )

    W1 = np.asarray(W1, f32)
    shared = {
        "w1y": lhsT_layout(dt * W1[:H]),
        "w2": lhsT_layout(np.asarray(W2, f32)),
        "wg": lhsT_layout(dt * np.asarray(Wg, f32)),
        "wh1": lhsT_layout(dt * np.asarray(Wh1, f32)),
        "w1b": np.ascontiguousarray(
            np.vstack([W1[H:], np.asarray(b1, f32)[None, :]])
        ),
        "b2x": np.vstack([np.zeros((C, H), f32), np.asarray(b2, f32)[None, :]]),
        "bgx": np.vstack([np.zeros((C, H), f32), np.asarray(bg, f32)[None, :]]),
        "wini": np.ascontiguousarray(
            np.vstack([np.asarray(W_init, f32), np.asarray(b_init, f32)[None, :]]) / dt
        ),
        "wh2": np.ascontiguousarray(
            np.asarray(Wh2, f32).reshape(KC, 128, O).transpose(1, 0, 2).reshape(128, KC * O)
        ),
        "bh1t": np.ascontiguousarray(np.asarray(bh1, f32).reshape(KC, 128).T),
        "bh2t": np.asarray(bh2, f32).reshape(O, 1),
    }

    coeffs = np.asarray(coeffs, f32)  # [B, T, C]
    dW = np.asarray(dW, f32)  # [NT, B, H]
    dw_scale = f32(0.5 * sq / dt)
    in_maps = []
    for c in range(NCORES):
        bs = slice(c * BL, (c + 1) * BL)
        xt = np.empty((T, C + 1, BL), f32)
        xt[:, :C, :] = coeffs[bs].transpose(1, 2, 0)
        xt[:, C, :] = 1.0
        dwc = (dW[:, bs, :] * dw_scale).transpose(0, 2, 1)  # [NT, H, BL]
        dwc = np.ascontiguousarray(
            dwc.reshape(NT, KC, 128, BL).transpose(0, 2, 1, 3).reshape(NT, 128, KC * BL)
        )
        in_maps.append({"xt": np.ascontiguousarray(xt), "dw": dwc, **shared})
    return in_maps


def kernel(**inputs):
    global _BUILT
    if _BUILT is None:
        _BUILT = _build_nc()
    nc = _BUILT
    in_maps = _prep_inputs(**inputs)
    res = run_bass_kernel_spmd(nc, in_maps, core_ids=list(range(NCORES)))
    out = np.empty((B, OUT_TIME, O), np.float32)
    for c, r in enumerate(res.results):
        # core out: [O, t*BL + b] -> [b, t, o]
        out[c * BL : (c + 1) * BL] = (
            r["out"].reshape(O, OUT_TIME, BL).transpose(2, 1, 0)
        )
    return out


# revision 9
# speedup vs baseline: 3.3233x; 3.3233x over previous
"""NeuralSDE forecasting kernel for 8x Trainium2 NeuronCores (Bass/Tile).

Data-parallel over batch B=256 across 8 cores (32 batch elems per core).
The per-core scan runs feature-major ("transposed"): state y.T lives in a
[128 partitions, 4*32] SBUF tile; column block k holds features
128k..128k+128 of the 32 local batch columns. Orientation: out = lhsT.T @
rhs with weight tiles stationary and the state as the 32-col moving
operand. No transposes anywhere.

Precision: the 255-step recurrence amplifies per-step rounding noise
~1000x, so bf16 operands fail (0.19 rel err) and fp32 matmuls are
weight-load bound (measured 7.3 ms: the fp32 Matmult reloads its 128-col
weight tile twice at ~225 ns). Instead each weight is split W = W_hi +
W_lo (both bf16) and each state operand y into y_hi + y_lo; the product
uses three bf16 matmuls (y_hi@W_hi + y_lo@W_hi + y_hi@W_lo, fp32 PSUM
accumulate) which restores ~fp32 accuracy (1.0e-3 measured end-to-end)
while loading weights via the 2x Fast-Weight-Load bf16 path. The two
W_hi products run as one N=64 matmul against packed [y_hi|y_lo]; the
W_lo product accumulates onto the lo half; a DVE add folds the halves.

sigmoid(x) = 0.5*(1+tanh(x/2)) keeps the scan on the Tanh ACT table only;
the 0.5 factors are folded into the host-prescaled dW. Biases ride an
appended ones-row of the control input (b1) or DVE bias adds (b2, bg).
"""

import os
import sys

sys.path.insert(0, "/opt/trn_rl_repo")

import numpy as np
import ml_dtypes

import concourse.bass as bass
import concourse.bacc as bacc
import concourse.mybir as mybir
import concourse.tile as tile
from concourse.bass_utils import run_bass_kernel_spmd

B, T, C, H, O = 256, 256, 32, 512, 32
OUT_TIME = 32
NCORES = 8
BL = B // NCORES  # 32 batch elements per core
NT = int(os.environ.get("BASS_NT", T - 1))  # 255 scan steps
SAVE0 = NT - OUT_TIME  # first step whose y_next lands in the output tail
KC = H // 128  # 4 feature chunks
F32 = mybir.dt.float32
BF16 = mybir.dt.bfloat16
BF = ml_dtypes.bfloat16

Tanh = mybir.ActivationFunctionType.Tanh
Relu = mybir.ActivationFunctionType.Relu
Identity = mybir.ActivationFunctionType.Identity

_BUILT = None


def _build_nc():
    nc = bacc.Bacc("TRN2", target_bir_lowering=False, debug=False)

    # --- DRAM I/O (per-core shards; weights replicated) ---
    d_xhl = nc.dram_tensor("xhl", [T, C + 1, 2 * BL], BF16, kind="ExternalInput")
    d_x0 = nc.dram_tensor("x0", [C + 1, BL], F32, kind="ExternalInput")
    d_dw = nc.dram_tensor("dw", [NT, 128, KC * BL], F32, kind="ExternalInput")
    wnames = ["w1y", "w2", "wg"]
    d_w = {
        (n, p): nc.dram_tensor(f"{n}_{p}", [128, KC * H], BF16, kind="ExternalInput")
        for n in wnames
        for p in ("hi", "lo")
    }
    d_w1b = {
        p: nc.dram_tensor(f"w1b_{p}", [C + 1, H], BF16, kind="ExternalInput")
        for p in ("hi", "lo")
    }
    d_wini = nc.dram_tensor("wini", [C + 1, H], F32, kind="ExternalInput")
    d_bb = nc.dram_tensor("biasb", [128, KC * BL], F32, kind="ExternalInput")
    d_bc = nc.dram_tensor("biasc", [128, KC * BL], F32, kind="ExternalInput")
    d_wh1 = nc.dram_tensor("wh1", [128, KC * H], F32, kind="ExternalInput")
    d_wh2 = nc.dram_tensor("wh2", [128, KC * O], F32, kind="ExternalInput")
    d_bh1 = nc.dram_tensor("bh1t", [128, KC], F32, kind="ExternalInput")
    d_bh2 = nc.dram_tensor("bh2t", [O, 1], F32, kind="ExternalInput")
    d_out = nc.dram_tensor("out", [O, OUT_TIME * BL], F32, kind="ExternalOutput")

    with tile.TileContext(nc) as tc:
        with (
            tc.tile_pool(name="const", bufs=1) as const,
            tc.tile_pool(name="xp", bufs=6) as xp,
            tc.tile_pool(name="dwp", bufs=6) as dwp,
            tc.tile_pool(name="yp", bufs=2) as yp,
            tc.tile_pool(name="tmp", bufs=3) as tmp,
            tc.tile_pool(name="pp", bufs=2, space="PSUM") as pp,
        ):
            # --- resident weights ---
            w_s = {}
            for key, d in d_w.items():
                w_s[key] = const.tile(
                    [128, KC * H], BF16, tag=f"{key[0]}_{key[1]}",
                    name=f"{key[0]}_{key[1]}_s",
                )
                nc.sync.dma_start(out=w_s[key][:], in_=d[:])
            w1b_s = {}
            for p, d in d_w1b.items():
                w1b_s[p] = const.tile([C + 1, H], BF16, tag=f"w1b{p}", name=f"w1b_{p}_s")
                nc.sync.dma_start(out=w1b_s[p][:], in_=d[:])
            wini = const.tile([C + 1, H], F32, tag="wini")
            biasb = const.tile([128, KC * BL], F32, tag="biasb")
            biasc = const.tile([128, KC * BL], F32, tag="biasc")
            wh1 = const.tile([128, KC * H], F32, tag="wh1")
            wh2 = const.tile([128, KC * O], F32, tag="wh2")
            bh1 = const.tile([128, KC], F32, tag="bh1")
            bh2 = const.tile([O, 1], F32, tag="bh2")
            x0 = const.tile([C + 1, BL], F32, tag="x0")
            slab = const.tile([128, OUT_TIME * 128], F32, tag="slab")
            rT = const.tile([128, KC * 1024], F32, tag="rT")
            outs = const.tile([O, OUT_TIME * BL], F32, tag="outs")
            for dst, src in [
                (wini, d_wini), (biasb, d_bb), (biasc, d_bc), (wh1, d_wh1),
                (wh2, d_wh2), (bh1, d_bh1), (bh2, d_bh2), (x0, d_x0),
            ]:
                nc.sync.dma_start(out=dst[:], in_=src[:])

            def wsl(n, p, k, m):  # lhsT tile (k, m) of weight n, part p
                return w_s[(n, p)][:, k * H + m * 128 : k * H + (m + 1) * 128]

            # double-bf16 matmul group: psum [128, 2*128]; m-chunk m uses
            # cols [64m, 64m+64): hi-sum in +0:32, lo-sum in +32:64.
            # rhs_hl: [128, KC*64] packed [y_hi|y_lo] per k-chunk (or the
            # [33, 64] x-tilde tile for the control-input part).
            def mm_group(ps, wname, rhs_hl, xt=None, w1bx=None):
                for m in range(KC):
                    hi = ps[:, m * 64 : m * 64 + 64]
                    lo = ps[:, m * 64 + 32 : m * 64 + 64]
                    first = True
                    if xt is not None:
                        nc.tensor.matmul(
                            hi, w1bx["hi"][:, m * 128 : (m + 1) * 128],
                            xt[:, 0 : 2 * BL], start=True, stop=False,
                        )
                        nc.tensor.matmul(
                            lo, w1bx["lo"][:, m * 128 : (m + 1) * 128],
                            xt[:, 0:BL], start=False, stop=False,
                        )
                        first = False
                    for k in range(KC):
                        nc.tensor.matmul(
                            hi, wsl(wname, "hi", k, m),
                            rhs_hl[:, k * 64 : k * 64 + 64],
                            start=first, stop=False,
                        )
                        first = False
                        nc.tensor.matmul(
                            lo, wsl(wname, "lo", k, m),
                            rhs_hl[:, k * 64 : k * 64 + 32],
                            start=False, stop=(k == KC - 1),
                        )

            def fold(dst, ps, extra=None):
                # dst[128, KC*32] = hi half + lo half (+ extra); DVE may read
                # only one PSUM operand per op, so fold in two steps.
                ps_r = ps[:].rearrange("p (m q) -> p m q", m=KC)
                dst_r = dst.rearrange("p (m b) -> p m b", m=KC)
                if extra is not None:
                    nc.vector.tensor_add(
                        dst_r, ps_r[:, :, 0:BL],
                        extra.rearrange("p (m b) -> p m b", m=KC),
                    )
                else:
                    nc.vector.tensor_copy(dst_r, ps_r[:, :, 0:BL])
                nc.vector.tensor_add(dst_r, dst_r, ps_r[:, :, BL : 2 * BL])

            def split_hl(hl_tile, src_ap):
                # hl_tile [128, KC*64]: per k-chunk hi in cols +0:32, lo +32:64
                hl_r = hl_tile[:].rearrange("p (k q) -> p k q", k=KC)
                src_r = src_ap.rearrange("p (k b) -> p k b", k=KC)
                nc.vector.tensor_copy(hl_r[:, :, 0:BL], src_r)
                nc.vector.tensor_sub(hl_r[:, :, BL : 2 * BL], src_r, hl_r[:, :, 0:BL])

            # --- z0 (fp32, one-off) ---
            ps0 = pp.tile([128, 2 * 128], F32, tag="psA")
            for m in range(KC):
                nc.tensor.matmul(
                    ps0[:, m * 64 : m * 64 + BL],
                    wini[:, m * 128 : (m + 1) * 128], x0[:],
                    start=True, stop=True,
                )
            y_t = yp.tile([128, KC * BL], F32, tag="y")
            ps0_r = ps0[:].rearrange("p (m q) -> p m q", m=KC)
            nc.vector.tensor_copy(
                y_t[:].rearrange("p (k b) -> p k b", k=KC), ps0_r[:, :, 0:BL]
            )
            y = y_t[:]
            yhl = tmp.tile([128, KC * 2 * BL], BF16, tag="yhl", name="yhl_init")
            split_hl(yhl, y)

            # --- scan ---
            for t in range(NT):
                xt_t = xp.tile([C + 1, 2 * BL], BF16, tag="x", name=f"x_{t}")
                nc.sync.dma_start(out=xt_t[:], in_=d_xhl[t])
                dw_t = dwp.tile([128, KC * BL], F32, tag="dw", name=f"dw_{t}")
                nc.sync.dma_start(out=dw_t[:], in_=d_dw[t])

                # h = tanh(y@W1y + x@W1x + b1)
                psA = pp.tile([128, 2 * 128], F32, tag="psA", name=f"psA_{t}")
                mm_group(psA, "w1y", yhl, xt=xt_t, w1bx=w1b_s)
                preA = tmp.tile([128, KC * BL], F32, tag="preA", name=f"preA_{t}")
                fold(preA[:], psA)
                h = tmp.tile([128, KC * BL], F32, tag="h", name=f"h_{t}")
                nc.scalar.activation(h[:], preA[:], Tanh)
                hhl = tmp.tile([128, KC * 2 * BL], BF16, tag="hhl", name=f"hhl_{t}")
                split_hl(hhl, h[:])

                # tau = tanh((y@Wg + bg)/2)  (sigmoid fold)
                psC = pp.tile([128, 2 * 128], F32, tag="psC", name=f"psC_{t}")
                mm_group(psC, "wg", yhl)
                preC = tmp.tile([128, KC * BL], F32, tag="preC", name=f"preC_{t}")
                fold(preC[:], psC, extra=biasc[:])
                tau = tmp.tile([128, KC * BL], F32, tag="tau", name=f"tau_{t}")
                nc.scalar.activation(tau[:], preC[:], Tanh, scale=0.5)
                # t1 = (tau + 1) * dw ;  dw pre-scaled by 0.5*sqrt(dt)/dt
                t1 = tmp.tile([128, KC * BL], F32, tag="t1", name=f"t1_{t}")
                nc.vector.scalar_tensor_tensor(
                    t1[:], tau[:], 1.0, dw_t[:],
                    mybir.AluOpType.add, mybir.AluOpType.mult,
                )
                yh2 = tmp.tile([128, KC * BL], F32, tag="yh2", name=f"yh2_{t}")
                nc.vector.tensor_add(yh2[:], y, t1[:])

                # f = tanh(h@W2 + b2)
                psB = pp.tile([128, 2 * 128], F32, tag="psB", name=f"psB_{t}")
                mm_group(psB, "w2", hhl)
                preB = tmp.tile([128, KC * BL], F32, tag="preB", name=f"preB_{t}")
                fold(preB[:], psB, extra=biasb[:])
                f = tmp.tile([128, KC * BL], F32, tag="f", name=f"f_{t}")
                nc.scalar.activation(f[:], preB[:], Tanh)

                # y_next = (y + t1) + f ; tail states land in the slab
                if t >= SAVE0:
                    y2 = slab[:, (t - SAVE0) * 128 : (t - SAVE0 + 1) * 128]
                else:
                    y2_t = yp.tile([128, KC * BL], F32, tag="y", name=f"y_{t}")
                    y2 = y2_t[:]
                nc.vector.tensor_add(y2, yh2[:], f[:])
                y = y2
                yhl = tmp.tile([128, KC * 2 * BL], BF16, tag="yhl", name=f"yhl_{t}")
                split_hl(yhl, y)

            # --- head (fp32): out = relu(z_tail@Wh1 + bh1) @ Wh2 + bh2 ---
            # slab columns: s*128 + k*32 + b  (s = tail step, k = feat chunk)
            slab_r = slab[:].rearrange(
                "p (s k b) -> p s k b", s=OUT_TIME, k=KC, b=BL
            )
            for m in range(KC):
                for hf in range(2):
                    ps1 = pp.tile([128, 512], F32, tag="psA", name=f"ps1_{m}_{hf}")
                    for k in range(KC):
                        nc.tensor.matmul(
                            ps1[:],
                            wh1[:, k * H + m * 128 : k * H + (m + 1) * 128],
                            slab_r[:, hf * 16 : (hf + 1) * 16, k, :],
                            start=(k == 0), stop=(k == KC - 1),
                        )
                    nc.scalar.activation(
                        rT[:, m * 1024 + hf * 512 : m * 1024 + (hf + 1) * 512],
                        ps1[:], Relu, bias=bh1[:, m : m + 1],
                    )
            for hf in range(2):
                ps2 = pp.tile([O, 512], F32, tag="psB", name=f"ps2_{hf}")
                for m in range(KC):
                    nc.tensor.matmul(
                        ps2[:],
                        wh2[:, m * O : (m + 1) * O],
                        rT[:, m * 1024 + hf * 512 : m * 1024 + (hf + 1) * 512],
                        start=(m == 0), stop=(m == KC - 1),
                    )
                nc.scalar.activation(
                    outs[:, hf * 512 : (hf + 1) * 512], ps2[:], Identity,
                    bias=bh2[:],
                )
            nc.sync.dma_start(out=d_out[:], in_=outs[:])

    nc.compile()
    return nc


def _split(w):
    hi = np.asarray(w, BF)
    lo = (np.asarray(w, np.float32) - hi.astype(np.float32)).astype(BF)
    return hi, lo


def _prep_inputs(times, coeffs, final_index, dW, W_init, b_init, W1, b1, W2,
                 b2, Wg, bg, Wh1, bh1, Wh2, bh2):
    f32 = np.float32
    times = np.asarray(times, f32)
    dt = f32(max(np.min(times[1:] - times[:-1]), f32(0.001)))
    sq = f32(np.sqrt(dt))

    def lhsT_layout(w):  # [H, H] -> [128, KC*H] with (k,m) tile at k*H+m*128
        return np.ascontiguousarray(
            np.asarray(w, f32).reshape(KC, 128, H).transpose(1, 0, 2).reshape(128, KC * H)
        )

    def bias_bcast(b):  # [H] -> [128, KC*BL] feature-major broadcast
        return np.ascontiguousarray(
            np.broadcast_to(
                np.asarray(b, f32).reshape(KC, 128).T[:, :, None], (128, KC, BL)
            ).reshape(128, KC * BL)
        )

    W1 = np.asarray(W1, f32)
    shared = {}
    for name, w in [("w1y", dt * W1[:H]), ("w2", np.asarray(W2, f32)),
                    ("wg", dt * np.asarray(Wg, f32))]:
        hi, lo = _split(lhsT_layout(w))
        shared[f"{name}_hi"] = hi
        shared[f"{name}_lo"] = lo
    w1b = np.vstack([W1[H:], np.asarray(b1, f32)[None, :]])
    shared["w1b_hi"], shared["w1b_lo"] = _split(w1b)
    shared["wini"] = np.ascontiguousarray(
        np.vstack([np.asarray(W_init, f32), np.asarray(b_init, f32)[None, :]]) / dt
    )
    shared["biasb"] = bias_bcast(b2)
    shared["biasc"] = bias_bcast(bg)
    shared["wh1"] = lhsT_layout(dt * np.asarray(Wh1, f32))
    shared["wh2"] = np.ascontiguousarray(
        np.asarray(Wh2, f32).reshape(KC, 128, O).transpose(1, 0, 2).reshape(128, KC * O)
    )
    shared["bh1t"] = np.ascontiguousarray(np.asarray(bh1, f32).reshape(KC, 128).T)
    shared["bh2t"] = np.asarray(bh2, f32).reshape(O, 1)

    coeffs = np.asarray(coeffs, f32)  # [B, T, C]
    dW = np.asarray(dW, f32)  # [NT_full, B, H]
    dw_scale = f32(0.5 * sq / dt)
    in_maps = []
    for c in range(NCORES):
        bs = slice(c * BL, (c + 1) * BL)
        xt = np.empty((T, C + 1, BL), f32)
        xt[:, :C, :] = coeffs[bs].transpose(1, 2, 0)
        xt[:, C, :] = 1.0
        xhi, xlo = _split(xt)
        xhl = np.empty((T, C + 1, 2 * BL), BF)
        xhl[:, :, :BL] = xhi
        xhl[:, :, BL:] = xlo
        dwc = (dW[:NT, bs, :] * dw_scale).transpose(0, 2, 1)  # [NT, H, BL]
        dwc = np.ascontiguousarray(
            dwc.reshape(NT, KC, 128, BL).transpose(0, 2, 1, 3).reshape(NT, 128, KC * BL)
        )
        in_maps.append(
            {"xhl": np.ascontiguousarray(xhl), "x0": np.ascontiguousarray(xt[0]),
             "dw": dwc, **shared}
        )
    return in_maps


def kernel(**inputs):
    global _BUILT
    if _BUILT is None:
        _BUILT = _build_nc()
    nc = _BUILT
    in_maps = _prep_inputs(**inputs)
    res = run_bass_kernel_spmd(nc, in_maps, core_ids=list(range(NCORES)))
    out = np.empty((B, OUT_TIME, O), np.float32)
    for c, r in enumerate(res.results):
        out[c * BL : (c + 1) * BL] = (
            r["out"].reshape(O, OUT_TIME, BL).transpose(2, 1, 0)
        )
    return out
